# revision 1
# baseline (speedup 1.0000x reference)
"""CoAttention kernel for Trainium2 (Bass/Tile), data-parallel over batch.

Problem: B=8, L=2048, D=1024 (dk = attention dim = 1024)
  k   = x @ Wk.T;  q_s = x @ Wq_self.T;  q_o = queries @ Wq_other.T;  v = values @ Wv.T
  scores_s = q_s @ k.T * scale (+penalty);  scores_o = q_o @ k.T * scale (+penalty)
  p_self = softmax(scores_s); p_other = softmax(scores_o)
  out = (p_self + p_other) @ v          # fused: one output matmul
  returns (out, p_self)

Sharding: batch b -> NeuronCore b (8 cores, no collectives).

Device-side layout trick: every matmul on the PE contracts along the partition
dim, so both operands need the contraction dim on partitions.  The host
pre-transposes x/queries/values (to [D, L]) and the 4 weights (to [D, A]) in
numpy, and pre-rounds everything to TF32 (fp32r) so the PE runs matmuls at
full rate (1 cycle/row; plain fp32 is 4x slower).

Per-core program phases:
  A  : kT[a,l] (resident SBUF) and q_sT[a,l] (spilled to DRAM) from xT
  A2 : q_oT[a,l] (spilled to DRAM) from queriesT
  P1 : per q-tile of 128 rows: scores (PSUM, fp32) -> exp (ACT, with row-sum
       accumulator) -> p_self = E_s/sum_s (DVE) -> P = p_self + E_o/sum_o
       (fused DVE op, rounded to fp32r) -> p_self, P to DRAM
  B  : v[l,a] (resident SBUF) from valuesT
  P2 : per q-tile: load P, PE-transpose 128x128 blocks -> out = P.T-blocks @ v
"""

import math
import sys

sys.path.insert(0, "/opt/trn_rl_repo")

import numpy as np

B, L, D, A = 8, 2048, 1024, 1024
DT = D // 128  # 8 contraction tiles for projections
AT = A // 128  # 8 attention-dim tiles
NLB = 4  # l superblocks of 512 in projections
NQT = L // 128  # 16 q tiles
NKB = 4  # key blocks of 512 in scores
SCALE = 1.0 / math.sqrt(D)

_CACHE: dict = {}


def tf32_round(x: np.ndarray) -> np.ndarray:
    """Round fp32 to TF32 (10-bit mantissa, RNE) — what the PE consumes in fp32r."""
    u = np.ascontiguousarray(x, dtype=np.float32).view(np.uint32)
    r = (u + np.uint32(0x0FFF) + ((u >> np.uint32(13)) & np.uint32(1))) & np.uint32(
        0xFFFFE000
    )
    return r.view(np.float32)


def _build(use_penalty: bool):
    import concourse.mybir as mybir
    from concourse import bacc
    from concourse.masks import make_identity
    from concourse.tile import TileContext

    f32 = mybir.dt.float32
    f32r = mybir.dt.float32r
    Exp = mybir.ActivationFunctionType.Exp

    nc = bacc.Bacc("TRN2", target_bir_lowering=False, debug=False)

    xT = nc.dram_tensor("xT", [D, L], f32r, kind="ExternalInput").ap()
    quT = nc.dram_tensor("queriesT", [D, L], f32r, kind="ExternalInput").ap()
    vaT = nc.dram_tensor("valuesT", [D, L], f32r, kind="ExternalInput").ap()
    wkT = nc.dram_tensor("wkT", [D, A], f32r, kind="ExternalInput").ap()
    wqsT = nc.dram_tensor("wqsT", [D, A], f32r, kind="ExternalInput").ap()
    wqoT = nc.dram_tensor("wqoT", [D, A], f32r, kind="ExternalInput").ap()
    wvT = nc.dram_tensor("wvT", [D, A], f32r, kind="ExternalInput").ap()
    pen = None
    if use_penalty:
        pen = nc.dram_tensor("pen", [1, L], f32r, kind="ExternalInput").ap()
    p_self = nc.dram_tensor("p_self", [L, L], f32, kind="ExternalOutput").ap()
    out = nc.dram_tensor("out", [L, A], f32, kind="ExternalOutput").ap()

    # [D, N] -> [128, DT, N] views (partition = d within tile)
    xT_r = xT.rearrange("(dt p) l -> p dt l", p=128)
    quT_r = quT.rearrange("(dt p) l -> p dt l", p=128)
    vaT_r = vaT.rearrange("(dt p) l -> p dt l", p=128)
    wkT_r = wkT.rearrange("(dt p) a -> p dt a", p=128)
    wqsT_r = wqsT.rearrange("(dt p) a -> p dt a", p=128)
    wqoT_r = wqoT.rearrange("(dt p) a -> p dt a", p=128)
    wvT_r = wvT.rearrange("(dt p) a -> p dt a", p=128)

    with TileContext(nc) as tc:
        with tc.tile_pool(name="dram", bufs=1, space="DRAM") as dram:
            qsT_d = dram.tile([A, L], f32r)  # q_self^T spill
            qoT_d = dram.tile([A, L], f32r)  # q_other^T spill
            P_d = dram.tile([L, L], f32r)  # combined probs spill
            qsT_dr = qsT_d.rearrange("(at p) q -> p at q", p=128)
            qoT_dr = qoT_d.rearrange("(at p) q -> p at q", p=128)

            if use_penalty:
                with tc.tile_pool(name="pen_pool", bufs=1) as pen_pool:
                    ones_f = pen_pool.tile([1, 128], f32)
                    nc.vector.memset(ones_f, 1.0)
                    ones_sb = pen_pool.tile([1, 128], f32r)
                    nc.vector.tensor_copy(ones_sb, ones_f)
                    pen_sb = pen_pool.tile([1, L], f32r)
                    nc.sync.dma_start(out=pen_sb, in_=pen)

            # ============ kT resident scope: phases A, A2, P1 ============
            with tc.tile_pool(name="k_res", bufs=1) as k_res:
                kT_sb = k_res.tile([128, AT, L], f32r)  # k^T resident, 8 MB

                # ---- Phase A: kT + q_sT from xT ----
                with (
                    tc.tile_pool(name="wA", bufs=1) as wA,
                    tc.tile_pool(name="xa_p", bufs=2) as xa_p,
                    tc.tile_pool(name="stage_p", bufs=3) as stage_p,
                    tc.tile_pool(name="psA", bufs=4, space="PSUM") as psA,
                ):
                    wk_sb = wA.tile([128, DT, A], f32r)
                    wqs_sb = wA.tile([128, DT, A], f32r)
                    nc.sync.dma_start(out=wk_sb, in_=wkT_r)
                    nc.sync.dma_start(out=wqs_sb, in_=wqsT_r)
                    for lb in range(NLB):
                        ls = slice(lb * 512, (lb + 1) * 512)
                        xa = xa_p.tile([128, DT, 512], f32r, tag="xa")
                        nc.sync.dma_start(out=xa, in_=xT_r[:, :, ls])
                        for at in range(AT):
                            psK = psA.tile([128, 512], f32, tag="pj")
                            for dt_ in range(DT):
                                nc.tensor.matmul(
                                    psK,
                                    wk_sb[:, dt_, at * 128 : (at + 1) * 128],
                                    xa[:, dt_, :],
                                    start=(dt_ == 0),
                                    stop=(dt_ == DT - 1),
                                )
                            nc.scalar.copy(kT_sb[:, at, ls], psK)
                            psQ = psA.tile([128, 512], f32, tag="pj")
                            for dt_ in range(DT):
                                nc.tensor.matmul(
                                    psQ,
                                    wqs_sb[:, dt_, at * 128 : (at + 1) * 128],
                                    xa[:, dt_, :],
                                    start=(dt_ == 0),
                                    stop=(dt_ == DT - 1),
                                )
                            qs_st = stage_p.tile([128, 512], f32r, tag="st")
                            nc.scalar.copy(qs_st, psQ)
                            nc.sync.dma_start(
                                out=qsT_d[at * 128 : (at + 1) * 128, ls], in_=qs_st
                            )

                # ---- Phase A2: q_oT from queriesT ----
                with (
                    tc.tile_pool(name="wA2", bufs=1) as wA2,
                    tc.tile_pool(name="qa_p", bufs=2) as qa_p,
                    tc.tile_pool(name="stage2_p", bufs=3) as stage2_p,
                    tc.tile_pool(name="psA2", bufs=4, space="PSUM") as psA2,
                ):
                    wqo_sb = wA2.tile([128, DT, A], f32r)
                    nc.sync.dma_start(out=wqo_sb, in_=wqoT_r)
                    for lb in range(NLB):
                        ls = slice(lb * 512, (lb + 1) * 512)
                        qa = qa_p.tile([128, DT, 512], f32r, tag="qa")
                        nc.sync.dma_start(out=qa, in_=quT_r[:, :, ls])
                        for at in range(AT):
                            psQ2 = psA2.tile([128, 512], f32, tag="pj2")
                            for dt_ in range(DT):
                                nc.tensor.matmul(
                                    psQ2,
                                    wqo_sb[:, dt_, at * 128 : (at + 1) * 128],
                                    qa[:, dt_, :],
                                    start=(dt_ == 0),
                                    stop=(dt_ == DT - 1),
                                )
                            qo_st = stage2_p.tile([128, 512], f32r, tag="st2")
                            nc.scalar.copy(qo_st, psQ2)
                            nc.sync.dma_start(
                                out=qoT_d[at * 128 : (at + 1) * 128, ls], in_=qo_st
                            )

                # ---- Pass 1: scores + softmax + P per q-tile ----
                with (
                    tc.tile_pool(name="qsl_p", bufs=2) as qsl_p,
                    tc.tile_pool(name="e_p", bufs=2) as e_p,
                    tc.tile_pool(name="pp_p", bufs=2) as pp_p,
                    tc.tile_pool(name="sum_p", bufs=4) as sum_p,
                    tc.tile_pool(name="psS", bufs=2, space="PSUM") as psS_p,
                ):
                    for qt in range(NQT):
                        qs_ = slice(qt * 128, (qt + 1) * 128)
                        qs_sl = qsl_p.tile([128, AT, 128], f32r, tag="qsl")
                        nc.sync.dma_start(out=qs_sl, in_=qsT_dr[:, :, qs_])
                        qo_sl = qsl_p.tile([128, AT, 128], f32r, tag="qol")
                        nc.sync.dma_start(out=qo_sl, in_=qoT_dr[:, :, qs_])

                        def scores(sl, e_tag):
                            psS = psS_p.tile([128, L], f32, tag="s")
                            for kb in range(NKB):
                                ks = slice(kb * 512, (kb + 1) * 512)
                                for at in range(AT):
                                    nc.tensor.matmul(
                                        psS[:, ks],
                                        sl[:, at, :],
                                        kT_sb[:, at, ks],
                                        start=(at == 0),
                                        stop=(at == AT - 1 and not use_penalty),
                                    )
                                if use_penalty:
                                    nc.tensor.matmul(
                                        psS[:, ks],
                                        ones_sb,
                                        pen_sb[:, ks],
                                        start=False,
                                        stop=True,
                                    )
                            e_sb = e_p.tile([128, L], f32, tag=e_tag, name=f"e_{e_tag}")
                            ssum = sum_p.tile(
                                [128, 1], f32, tag=f"sum{e_tag}", name=f"ssum_{e_tag}"
                            )
                            nc.scalar.activation(
                                out=e_sb, in_=psS, func=Exp, scale=SCALE, accum_out=ssum
                            )
                            recip = sum_p.tile(
                                [128, 1], f32, tag=f"rec{e_tag}", name=f"recip_{e_tag}"
                            )
                            nc.vector.reciprocal(recip, ssum)
                            return e_sb, recip

                        e_s, rec_s = scores(qs_sl, "es")
                        e_o, rec_o = scores(qo_sl, "eo")

                        psf = pp_p.tile([128, L], f32, tag="psf")
                        nc.vector.tensor_scalar_mul(psf, e_s, rec_s)
                        nc.sync.dma_start(out=p_self[qs_, :], in_=psf)

                        pcomb = pp_p.tile([128, L], f32r, tag="pc")
                        nc.vector.scalar_tensor_tensor(
                            out=pcomb,
                            in0=e_o,
                            scalar=rec_o,
                            in1=psf,
                            op0=mybir.AluOpType.mult,
                            op1=mybir.AluOpType.add,
                        )
                        nc.sync.dma_start(out=P_d[qs_, :], in_=pcomb)

            # ============ v resident scope: phases B, P2 ============
            with tc.tile_pool(name="v_res", bufs=1) as v_res:
                v_sb = v_res.tile([128, NQT, A], f32r)  # v resident, 8 MB

                # ---- Phase B: v from valuesT ----
                with (
                    tc.tile_pool(name="wB", bufs=1) as wB,
                    tc.tile_pool(name="va_p", bufs=2) as va_p,
                    tc.tile_pool(name="psB", bufs=4, space="PSUM") as psB,
                ):
                    wv_sb = wB.tile([128, DT, A], f32r)
                    nc.sync.dma_start(out=wv_sb, in_=wvT_r)
                    for lb in range(NLB):
                        ls = slice(lb * 512, (lb + 1) * 512)
                        va = va_p.tile([128, DT, 512], f32r, tag="va")
                        nc.sync.dma_start(out=va, in_=vaT_r[:, :, ls])
                        for lt in range(4):  # 4 l-tiles of 128 per superblock
                            gt = lb * 4 + lt
                            for ab in range(2):
                                psV = psB.tile([128, 512], f32, tag="pv")
                                for dt_ in range(DT):
                                    nc.tensor.matmul(
                                        psV,
                                        va[:, dt_, lt * 128 : (lt + 1) * 128],
                                        wv_sb[:, dt_, ab * 512 : (ab + 1) * 512],
                                        start=(dt_ == 0),
                                        stop=(dt_ == DT - 1),
                                    )
                                nc.scalar.copy(
                                    v_sb[:, gt, ab * 512 : (ab + 1) * 512], psV
                                )

                # ---- Pass 2: out = P-blocks^T @ v ----
                with (
                    tc.tile_pool(name="ident_p", bufs=1) as ident_p,
                    tc.tile_pool(name="pt_in", bufs=3) as pt_in,
                    tc.tile_pool(name="ptb", bufs=2) as ptb,
                    tc.tile_pool(name="outs", bufs=2) as outs,
                    tc.tile_pool(name="psT", bufs=4, space="PSUM") as psT,
                    tc.tile_pool(name="psO", bufs=4, space="PSUM") as psO,
                ):
                    ident_f = ident_p.tile([128, 128], f32)
                    make_identity(nc, ident_f)
                    ident = ident_p.tile([128, 128], f32r)
                    nc.vector.tensor_copy(ident, ident_f)

                    for qt in range(NQT):
                        qs_ = slice(qt * 128, (qt + 1) * 128)
                        p_t = pt_in.tile([128, L], f32r, tag="pt")
                        nc.sync.dma_start(out=p_t, in_=P_d[qs_, :])
                        ptr = ptb.tile([128, L], f32r, tag="ptr")
                        for kt in range(NQT):
                            kslc = slice(kt * 128, (kt + 1) * 128)
                            tp_ps = psT.tile([128, 128], f32r, tag="tp")
                            nc.tensor.transpose(tp_ps, p_t[:, kslc], ident)
                            nc.vector.tensor_copy(ptr[:, kslc], tp_ps)
                        out_sb = outs.tile([128, A], f32, tag="ou")
                        for ab in range(2):
                            psOt = psO.tile([128, 512], f32, tag="po")
                            for kt in range(NQT):
                                nc.tensor.matmul(
                                    psOt,
                                    ptr[:, kt * 128 : (kt + 1) * 128],
                                    v_sb[:, kt, ab * 512 : (ab + 1) * 512],
                                    start=(kt == 0),
                                    stop=(kt == NQT - 1),
                                )
                            nc.scalar.copy(out_sb[:, ab * 512 : (ab + 1) * 512], psOt)
                        nc.sync.dma_start(out=out[qs_, :], in_=out_sb)

    nc.compile()
    return nc


def _get_program(use_penalty: bool):
    key = ("prog", use_penalty)
    if key not in _CACHE:
        _CACHE[key] = _build(use_penalty)
    return _CACHE[key]


def _prep_inputs(x, queries, values, attention_mask, Wk, Wq_self, Wq_other, Wv):
    """Host-side: TF32-round + transpose, build per-core input maps."""
    x = np.asarray(x, dtype=np.float32)
    queries = np.asarray(queries, dtype=np.float32)
    values = np.asarray(values, dtype=np.float32)
    attention_mask = np.asarray(attention_mask, dtype=np.float32)

    use_penalty = not bool(np.all(attention_mask == 1.0))

    wk_t = np.ascontiguousarray(tf32_round(np.asarray(Wk, np.float32)).T)
    wqs_t = np.ascontiguousarray(tf32_round(np.asarray(Wq_self, np.float32)).T)
    wqo_t = np.ascontiguousarray(tf32_round(np.asarray(Wq_other, np.float32)).T)
    wv_t = np.ascontiguousarray(tf32_round(np.asarray(Wv, np.float32)).T)

    in_maps = []
    for b in range(B):
        m = {
            "xT": np.ascontiguousarray(tf32_round(x[b]).T),
            "queriesT": np.ascontiguousarray(tf32_round(queries[b]).T),
            "valuesT": np.ascontiguousarray(tf32_round(values[b]).T),
            "wkT": wk_t,
            "wqsT": wqs_t,
            "wqoT": wqo_t,
            "wvT": wv_t,
        }
        if use_penalty:
            # penalty pre-divided by SCALE because exp applies scale to the sum
            m["pen"] = tf32_round(
                ((1.0 - attention_mask[b]) * (-100000.0 / SCALE)).reshape(1, L)
            )
        in_maps.append(m)
    return use_penalty, in_maps


def kernel(x, queries, values, attention_mask, Wk, Wq_self, Wq_other, Wv, *, _trace=False):
    from concourse import bass_utils

    use_penalty, in_maps = _prep_inputs(
        x, queries, values, attention_mask, Wk, Wq_self, Wq_other, Wv
    )
    nc = _get_program(use_penalty)
    res = bass_utils.run_bass_kernel_spmd(
        nc, in_maps, core_ids=list(range(B)), trace=_trace
    )
    out = np.stack([res.results[b]["out"] for b in range(B)])
    p_self = np.stack([res.results[b]["p_self"] for b in range(B)])
    if _trace:
        _CACHE["last_results"] = res
    return out, p_self
